# revision 15
# baseline (speedup 1.0000x reference)
"""DCNv2 (modulated deformable conv) + BN + SiLU Trainium2 Bass kernel.

Problem: nn_DeformConv_58935541236111
  x[4,256,64,64]: offset/mask conv (3x3, 256->27, +bias) -> clamp/sigmoid ->
  bilinear sampling -> einsum over (C1*KK) with w_dcn -> BN -> SiLU.

Sharding: 8 cores = batch (4) x row-half (2); core computes
out[b, :, 32r:32r+32, :]  (2048 pixels, row-major p = h_loc*64 + w).

v3 design (channel-major, no DMA weight broadcast):
  - x staged as row-pair tokens in SBUF: token (x,y) = 512 bf16 =
    [256ch @ (y,x), 256ch @ (y+1,x)] over a 48-row x 80-col zero-padded
    grid.  Bilinear corners of one sample = tokens idx and idx+48.
  - dma_gather (SBUF source, transpose=True, 1KB tokens): per (k, half)
    one gather of 2048 idxs -> gh[128ch, 4, 2048] channel-major
    (j = row y0/y1 x ct; i<1024 col x0, i>=1024 col x0+1).
  - corner weights: computed per-pixel on 128 partitions (cheap chain),
    staged to DRAM, reloaded as [36, 2, 1024] rows, then broadcast
    across partitions by the PE (ones[1,128] ^T @ w_row[1,1024] into
    PSUM bf16) and copied to SBUF by ACT.  Zero DMA broadcast bytes.
  - lerp: 12 DVE + 2 Pool tensor_tensor ops per (k, half) produce two
    corner-pair partial cols (colA = w00*v00 + w01*v01, colB = w10*v10
    + w11*v11); the final pair-sum is absorbed into a doubled einsum
    (PSUM accumulates colA and colB matmuls).
  - BN+SiLU per pixel-half; store row-major (no permutation).
  - pipeline: 18 (half,k) steps, gather i ahead of compute i-2.
"""

import numpy as np
import ml_dtypes

B, C1, C2, H, W = 4, 256, 256, 64, 64
MAX_OFF = 6.0
BN_EPS = 1e-5

NCORES = 8
HL = 32                 # rows per core
P = HL * W              # 2048 pixels / core
PH = P // 2             # pixels per half
PAD = 8
GRID_Y = HL + 2 * PAD   # 48
GRID_X = W + 2 * PAD    # 80
NTOK = GRID_X * GRID_Y  # 3840 = 128 * 30
NIDX = 2048             # idxs per gather (1024 px * 2 columns)

BF16 = ml_dtypes.bfloat16


def _build_nc():
    import concourse.bacc as bacc
    import concourse.mybir as mybir
    import concourse.tile as tile

    f32 = mybir.dt.float32
    bf16 = mybir.dt.bfloat16
    i16 = mybir.dt.int16
    i32 = mybir.dt.int32
    AF = mybir.ActivationFunctionType
    OP = mybir.AluOpType

    nc = bacc.Bacc("TRN2", target_bir_lowering=False, debug=False)

    x_pairs_d = nc.dram_tensor("x_pairs", [NTOK, 512], bf16, kind="ExternalInput")
    x_conv_d = nc.dram_tensor("x_conv", [2, 128, 34, 66], bf16, kind="ExternalInput")
    w_om_d = nc.dram_tensor("w_om", [9, 2, 128, 27], bf16, kind="ExternalInput")
    w_dcn_d = nc.dram_tensor("w_dcn", [9, 2, 2, 128, 128], bf16, kind="ExternalInput")
    base_y_d = nc.dram_tensor("base_y", [128, 9, 16], f32, kind="ExternalInput")
    base_x_d = nc.dram_tensor("base_x", [128, 9, 16], f32, kind="ExternalInput")
    bias_y_d = nc.dram_tensor("bias_y", [128, 9, 16], f32, kind="ExternalInput")
    bias_x_d = nc.dram_tensor("bias_x", [128, 9, 16], f32, kind="ExternalInput")
    bias_m_d = nc.dram_tensor("bias_m", [128, 9, 16], f32, kind="ExternalInput")
    ident_d = nc.dram_tensor("ident", [128, 128], bf16, kind="ExternalInput")
    sel_d = nc.dram_tensor("sel", [4, 512], bf16, kind="ExternalInput")
    bn_d = nc.dram_tensor("bn", [4, 128, 2], f32, kind="ExternalInput")
    out_d = nc.dram_tensor("out", [2, 128, P], f32, kind="ExternalOutput")
    # idx staging, read-order layout: [hs][q][k][col][sl][ph]
    stage_d = nc.dram_tensor("stage", [2, 16, 9, 2, 8, 8], i16)
    # weight staging: [hs][(row col k)][(sl pp)]
    wstage_d = nc.dram_tensor("wstage", [2, 4, 9, 1024], bf16)

    with tile.TileContext(nc) as tc:
        with (
            tc.tile_pool(name="pers", bufs=1) as pers,
            tc.tile_pool(name="chainp", bufs=1) as chainp,
            tc.tile_pool(name="gpool", bufs=2) as gpool,
            tc.tile_pool(name="lerpp", bufs=2) as lerpp,
            tc.tile_pool(name="colp", bufs=2) as colp,
            tc.tile_pool(name="wsbp", bufs=2) as wsbp,
            tc.tile_pool(name="outp", bufs=2) as outp,
        ):
            # ---------------- persistent tensors / loads ----------------
            xc = pers.tile([128, 2, 34, 66], bf16)
            nc.scalar.dma_start(xc[:], x_conv_d[:].rearrange("c p a b -> p c a b"))
            w_om = pers.tile([128, 9, 2, 27], bf16)
            nc.scalar.dma_start(w_om[:], w_om_d[:].rearrange("k c p o -> p k c o"))
            ident = pers.tile([128, 128], bf16)
            nc.scalar.dma_start(ident[:], ident_d[:])
            x_sb = pers.tile([128, 30, 512], bf16)
            nc.sync.dma_start(x_sb[:], x_pairs_d[:].rearrange("(r p) e -> p r e", p=128))
            wd = pers.tile([128, 9, 2, 2, 128], bf16)
            nc.sync.dma_start(wd[:], w_dcn_d[:].rearrange("k c o p q -> p k c o q"))
            base_y = pers.tile([128, 9, 16], f32)
            nc.sync.dma_start(base_y[:], base_y_d[:])
            base_x = pers.tile([128, 9, 16], f32)
            nc.sync.dma_start(base_x[:], base_x_d[:])
            bias_y = pers.tile([128, 9, 16], f32)
            nc.sync.dma_start(bias_y[:], bias_y_d[:])
            bias_x = pers.tile([128, 9, 16], f32)
            nc.sync.dma_start(bias_x[:], bias_x_d[:])
            bias_m = pers.tile([128, 9, 16], f32)
            nc.sync.dma_start(bias_m[:], bias_m_d[:])
            bn_in = pers.tile([128, 4, 2], f32)
            nc.sync.dma_start(bn_in[:], bn_d[:].rearrange("a p b -> p a b"))

            bn_s = pers.tile([128, 2], f32)
            bn_o = pers.tile([128, 2], f32)
            wrap_rep = pers.tile([128, 2, 9, 128], i16)
            # [p, hs, row(y), col(x), k, sl] - bf16 corner weights
            w4 = pers.tile([128, 2, 2, 2, 9, 8], bf16)
            idx_all = pers.tile([128, 2, 9, 2, 8], i16)   # [p, hs, k, col, sl]
            w_mov = pers.tile([4, 9, 2, 1024], bf16)      # [cr, k, hs, px]
            sel = pers.tile([4, 512], bf16)
            nc.scalar.dma_start(sel[:], sel_d[:])

            def ct_(name, dtype=f32, shape=(128, 9, 8)):
                return chainp.tile(list(shape), dtype, tag=name, name=name)

            def chain_half(hs, omT):
                hsl = slice(hs * 8, hs * 8 + 8)
                omr = omT[:].rearrange("p s o -> p o s")
                dy = ct_("dy"); dx = ct_("dx"); mm = ct_("mm")
                nc.vector.tensor_copy(dy[:], omr[:, 0:18:2, :])
                nc.vector.tensor_copy(dx[:], omr[:, 1:18:2, :])
                nc.vector.tensor_copy(mm[:], omr[:, 18:27, :])
                nc.vector.tensor_tensor(dy[:], dy[:], bias_y[:, :, hsl], OP.add)
                nc.vector.tensor_tensor(dx[:], dx[:], bias_x[:, :, hsl], OP.add)
                nc.vector.tensor_tensor(mm[:], mm[:], bias_m[:, :, hsl], OP.add)
                nc.vector.tensor_scalar(dy[:], dy[:], MAX_OFF, -MAX_OFF, OP.min, OP.max)
                nc.vector.tensor_scalar(dx[:], dx[:], MAX_OFF, -MAX_OFF, OP.min, OP.max)
                py_ = ct_("py_"); px_ = ct_("px_")
                nc.vector.tensor_tensor(py_[:], dy[:], base_y[:, :, hsl], OP.add)
                nc.vector.tensor_tensor(px_[:], dx[:], base_x[:, :, hsl], OP.add)
                iy = ct_("iy", i32); t0 = ct_("t0"); t1 = ct_("t1")
                y0f = ct_("y0f"); x0f = ct_("x0f"); ly = ct_("ly"); lx = ct_("lx")
                # floor via convert + fixup (robust to converter rounding mode)
                nc.vector.tensor_copy(iy[:], py_[:])
                nc.vector.tensor_copy(y0f[:], iy[:])
                nc.vector.tensor_tensor(t0[:], y0f[:], py_[:], OP.is_gt)
                nc.vector.tensor_tensor(y0f[:], y0f[:], t0[:], OP.subtract)
                nc.vector.tensor_tensor(ly[:], py_[:], y0f[:], OP.subtract)
                nc.vector.tensor_copy(iy[:], px_[:])
                nc.vector.tensor_copy(x0f[:], iy[:])
                nc.vector.tensor_tensor(t1[:], x0f[:], px_[:], OP.is_gt)
                nc.vector.tensor_tensor(x0f[:], x0f[:], t1[:], OP.subtract)
                nc.vector.tensor_tensor(lx[:], px_[:], x0f[:], OP.subtract)
                # token index = x0*48 + y0 ; second column at +48
                nc.vector.tensor_scalar(t0[:], x0f[:], float(GRID_Y), None, OP.mult)
                nc.vector.tensor_tensor(t0[:], t0[:], y0f[:], OP.add)
                nc.vector.tensor_copy(idx_all[:, hs, :, 0], t0[:])
                nc.vector.tensor_scalar(t0[:], t0[:], float(GRID_Y), None, OP.add)
                nc.vector.tensor_copy(idx_all[:, hs, :, 1], t0[:])
                # corner weights (mask folded in)
                msk = ct_("msk")
                nc.scalar.activation(msk[:], mm[:], AF.Sigmoid)
                oly = ct_("oly"); olx = ct_("olx")
                nc.vector.tensor_scalar(oly[:], ly[:], -1.0, 1.0, OP.mult, OP.add)
                nc.vector.tensor_scalar(olx[:], lx[:], -1.0, 1.0, OP.mult, OP.add)
                wyt = ct_("wyt"); wyb = ct_("wyb")
                nc.vector.tensor_tensor(wyt[:], oly[:], msk[:], OP.mult)
                nc.vector.tensor_tensor(wyb[:], ly[:], msk[:], OP.mult)
                nc.vector.tensor_tensor(w4[:, hs, 0, 0], wyt[:], olx[:], OP.mult)
                nc.vector.tensor_tensor(w4[:, hs, 0, 1], wyt[:], lx[:], OP.mult)
                nc.vector.tensor_tensor(w4[:, hs, 1, 0], wyb[:], olx[:], OP.mult)
                nc.vector.tensor_tensor(w4[:, hs, 1, 1], wyb[:], lx[:], OP.mult)

            def wraps_half(hs):
                # indices: SBUF -> DRAM (read-order) -> 8 replicated reads
                for ph in range(8):
                    eng = nc.sync if ph % 2 == 0 else nc.scalar
                    eng.dma_start(
                        stage_d[hs, :, :, :, :, ph],
                        idx_all[ph * 16:(ph + 1) * 16, hs],
                    )
                for g in range(8):
                    eng = nc.sync if g % 2 == 0 else nc.scalar
                    eng.dma_start(
                        wrap_rep[g * 16:(g + 1) * 16, hs],
                        stage_d[hs].rearrange("q k col sl ph -> q k (col sl ph)"),
                    )
                # weights: SBUF -> DRAM (row-col-k x sl-pp) -> [36, 1024] rows
                nc.sync.dma_start(
                    wstage_d[hs].rearrange(
                        "(row col) k (sl pp) -> pp row col k sl",
                        row=2, col=2, sl=8, pp=128),
                    w4[:, hs],
                )
                nc.scalar.dma_start(
                    w_mov[:, :, hs],
                    wstage_d[hs],
                )

            gh_tiles = {}

            def gather(hs, k):
                gh = gpool.tile([128, 4, NIDX], bf16, tag="gh", name=f"gh{hs}_{k}")
                gh_tiles[(hs, k)] = gh
                nc.gpsimd.dma_gather(
                    gh[:],
                    x_sb[:].rearrange("p r e -> p (r e)"),
                    wrap_rep[:, hs, k],
                    NIDX,
                    NIDX,
                    512,
                    transpose=True,
                    sbuf_tokens_per_rank=128,
                    sbuf_free_dim_per_rank=1024,
                    single_packet=False,
                )

            # ---------------- phase A: conv + chain (psum scope A) -------
            with tc.tile_pool(name="psA", bufs=1, space="PSUM") as psA:
                # BN constants
                tv = chainp.tile([128, 2], f32, tag="tv", name="tv")
                nc.vector.tensor_scalar(tv[:], bn_in[:, 3], BN_EPS, None, OP.add)
                nc.scalar.sqrt(tv[:], tv[:])
                nc.vector.reciprocal(tv[:], tv[:])
                nc.vector.tensor_tensor(bn_s[:], bn_in[:, 0], tv[:], OP.mult)
                nc.vector.tensor_tensor(bn_o[:], bn_in[:, 2], bn_s[:], OP.mult)
                nc.vector.tensor_tensor(bn_o[:], bn_in[:, 1], bn_o[:], OP.subtract)

                def conv_half(hs):
                    om_ps = psA.tile([27, PH], f32, tag="om", name=f"om{hs}")
                    for ky in range(3):
                        for kx in range(3):
                            k = ky * 3 + kx
                            for ctile in range(2):
                                for n in range(2):
                                    nc.tensor.matmul(
                                        om_ps[:, n * 512:(n + 1) * 512],
                                        w_om[:, k, ctile],
                                        xc[:, ctile,
                                           hs * 16 + n * 8 + ky: hs * 16 + n * 8 + ky + 8,
                                           kx: kx + 64],
                                        start=(k == 0 and ctile == 0),
                                        stop=(k == 8 and ctile == 1),
                                    )
                    om_sb = chainp.tile([27, PH], bf16, tag="om_sb", name=f"om_sb{hs}")
                    nc.scalar.copy(om_sb[:], om_ps[:])
                    omT_ps = psA.tile([128, 8 * 28], bf16, tag="omt", name=f"omt{hs}")
                    for c8 in range(8):
                        nc.tensor.transpose(
                            omT_ps[:, c8 * 28:c8 * 28 + 27],
                            om_sb[:, c8 * 128:(c8 + 1) * 128],
                            ident[:27, :27],
                        )
                    omT = chainp.tile([128, 8, 27], f32, tag=f"omT{hs}", name=f"omT{hs}")
                    nc.scalar.copy(
                        omT[:],
                        omT_ps[:].rearrange("p (a b) -> p a b", a=8)[:, :, 0:27],
                    )
                    return omT

                omT0 = conv_half(0)
                chain_half(0, omT0)
                wraps_half(0)
                gather(0, 0)
                gather(0, 1)
                omT1 = conv_half(1)
                chain_half(1, omT1)
                wraps_half(1)

            # ---------------- phase B: main loop (psum scope B) ----------
            sched = [(0, k) for k in range(9)] + [(1, k) for k in range(9)]
            with tc.tile_pool(name="psB", bufs=1, space="PSUM") as psB:
                out_ps = [psB.tile([128, PH], f32, tag=f"o{ot}", name=f"out_ps{ot}")
                          for ot in range(2)]

                def compute(hs, k):
                    gh = gh_tiles.pop((hs, k))
                    # PE: broadcast the 4 corner-weight rows across partitions
                    # (two corner-pair rounds; psum f32, ACT copies to bf16)
                    w_sb = wsbp.tile([128, 4, PH], bf16, tag="wsb",
                                     name=f"wsb{hs}_{k}")
                    for pair in range(2):
                        w_ps = psB.tile([128, 2, PH], f32, tag="wps",
                                        name=f"wps{hs}_{k}_{pair}")
                        for j in range(2):
                            cr = pair * 2 + j
                            for n in range(2):
                                nc.tensor.matmul(
                                    w_ps[:, j, n * 512:(n + 1) * 512],
                                    sel[:, cr * 128:(cr + 1) * 128],
                                    w_mov[:, k, hs, n * 512:(n + 1) * 512],
                                    start=True, stop=True)
                        nc.scalar.copy(w_sb[:, pair * 2:pair * 2 + 2], w_ps[:])
                    # lerp: colA = w00*v00 + w01*v01, colB = w10*v10 + w11*v11
                    colA = colp.tile([128, 2, PH], bf16, tag="colA",
                                     name=f"colA{hs}_{k}")
                    colB = colp.tile([128, 2, PH], bf16, tag="colB",
                                     name=f"colB{hs}_{k}")
                    ta = lerpp.tile([128, PH], bf16, tag="ta", name=f"ta{hs}_{k}")
                    tp = lerpp.tile([128, PH], bf16, tag="tp", name=f"tp{hs}_{k}")
                    for ctile in range(2):
                        cA = colA[:, ctile]
                        cB = colB[:, ctile]
                        v00 = gh[:, ctile, 0:PH]
                        v01 = gh[:, ctile, PH:2 * PH]
                        v10 = gh[:, 2 + ctile, 0:PH]
                        v11 = gh[:, 2 + ctile, PH:2 * PH]
                        nc.vector.tensor_tensor(cA, v00, w_sb[:, 0], OP.mult)
                        nc.vector.tensor_tensor(ta[:], v01, w_sb[:, 1], OP.mult)
                        nc.vector.tensor_tensor(cA, cA, ta[:], OP.add)
                        nc.vector.tensor_tensor(cB, v10, w_sb[:, 2], OP.mult)
                        nc.gpsimd.tensor_tensor(tp[:], v11, w_sb[:, 3], OP.mult)
                        nc.vector.tensor_tensor(cB, cB, tp[:], OP.add)
                    for ctile in range(2):
                        for ot in range(2):
                            for n in range(2):
                                sl = slice(n * 512, (n + 1) * 512)
                                nc.tensor.matmul(
                                    out_ps[ot][:, sl],
                                    wd[:, k, ctile, ot],
                                    colA[:, ctile, sl],
                                    start=(k == 0 and ctile == 0),
                                    stop=False,
                                )
                                nc.tensor.matmul(
                                    out_ps[ot][:, sl],
                                    wd[:, k, ctile, ot],
                                    colB[:, ctile, sl],
                                    start=False,
                                    stop=(k == 8 and ctile == 1),
                                )

                def bn_half(hs):
                    for ot in range(2):
                        yv = outp.tile([128, PH], f32, tag="yv", name=f"yv{hs}_{ot}")
                        sg = outp.tile([128, PH], f32, tag="sg", name=f"sg{hs}_{ot}")
                        osb = outp.tile([128, PH], f32, tag="osb", name=f"osb{hs}_{ot}")
                        nc.vector.tensor_scalar(
                            yv[:], out_ps[ot][:],
                            bn_s[:, ot:ot + 1], bn_o[:, ot:ot + 1],
                            OP.mult, OP.add)
                        nc.scalar.activation(sg[:], yv[:], AF.Sigmoid)
                        nc.vector.tensor_tensor(osb[:], yv[:], sg[:], OP.mult)
                        nc.sync.dma_start(out_d[ot, :, hs * PH:(hs + 1) * PH], osb[:])

                for i in range(2, 18):
                    hs, k = sched[i - 2]
                    compute(hs, k)
                    if k == 8:
                        bn_half(hs)
                    gather(*sched[i])
                compute(*sched[16])
                compute(*sched[17])
                bn_half(1)

    nc.compile()
    return nc


def _prep_core_inputs(inputs, b, r):
    x = np.asarray(inputs["x"])
    w_om = np.asarray(inputs["w_om"])
    b_om = np.asarray(inputs["b_om"])
    w_dcn = np.asarray(inputs["w_dcn"])
    h0 = HL * r

    # padded spatial-major grid [49 rows, 80 cols, 256 ch] (extra row so the
    # y+1 half of the last token row reads zeros)
    xp = np.zeros((GRID_Y + 1, GRID_X, 256), dtype=BF16)
    y_lo, y_hi = max(0, h0 - PAD), min(H, h0 + HL + PAD)
    xp[y_lo - (h0 - PAD):y_hi - (h0 - PAD), PAD:PAD + W, :] = (
        x[b][:, y_lo:y_hi, :].transpose(1, 2, 0).astype(BF16)
    )
    pair = np.concatenate([xp[0:GRID_Y], xp[1:GRID_Y + 1]], axis=2)  # [48, 80, 512]
    x_pairs = np.ascontiguousarray(pair.swapaxes(0, 1).reshape(NTOK, 512))

    xcv = np.zeros((256, 34, 66), dtype=BF16)
    r_lo, r_hi = max(0, h0 - 1), min(H, h0 + 33)
    xcv[:, r_lo - (h0 - 1):r_hi - (h0 - 1), 1:65] = x[b][:, r_lo:r_hi, :].astype(BF16)
    x_conv = np.ascontiguousarray(xcv.reshape(2, 128, 34, 66))

    wl = np.zeros((9, 2, 128, 27), dtype=BF16)
    for ky in range(3):
        for kx in range(3):
            k = ky * 3 + kx
            for ctile in range(2):
                wl[k, ctile] = w_om[:, ctile * 128:(ctile + 1) * 128, ky, kx].T.astype(BF16)

    wdl = np.zeros((9, 2, 2, 128, 128), dtype=BF16)
    wr = w_dcn.reshape(C2, C1, 9)
    for k in range(9):
        for ctile in range(2):
            for ot in range(2):
                wdl[k, ctile, ot] = wr[ot * 128:(ot + 1) * 128,
                                       ctile * 128:(ctile + 1) * 128, k].T.astype(BF16)

    # pixel p = fl*128 + part ; h_loc = p//64, w = p%64 (row-major)
    p_ = np.arange(128)[:, None, None]
    k_ = np.arange(9)[None, :, None]
    fl = np.arange(16)[None, None, :]
    pix = fl * 128 + p_
    h_loc = pix // W
    w_pix = pix % W
    ky_ = k_ // 3
    kx_ = k_ % 3
    base_y = np.broadcast_to(h_loc + ky_ - 1 + PAD, (128, 9, 16)).astype(np.float32)
    base_x = np.broadcast_to(w_pix + kx_ - 1 + PAD, (128, 9, 16)).astype(np.float32)
    bias_y = np.broadcast_to(b_om[0:18:2][None, :, None], (128, 9, 16)).astype(np.float32)
    bias_x = np.broadcast_to(b_om[1:18:2][None, :, None], (128, 9, 16)).astype(np.float32)
    bias_m = np.broadcast_to(b_om[18:27][None, :, None], (128, 9, 16)).astype(np.float32)

    bn = np.stack([
        np.asarray(inputs["bn_gamma"]).reshape(2, 128).T,
        np.asarray(inputs["bn_beta"]).reshape(2, 128).T,
        np.asarray(inputs["bn_mean"]).reshape(2, 128).T,
        np.asarray(inputs["bn_var"]).reshape(2, 128).T,
    ], axis=0).astype(np.float32)

    return {
        "x_pairs": x_pairs,
        "x_conv": x_conv,
        "w_om": wl,
        "w_dcn": wdl,
        "base_y": np.ascontiguousarray(base_y),
        "base_x": np.ascontiguousarray(base_x),
        "bias_y": np.ascontiguousarray(bias_y),
        "bias_x": np.ascontiguousarray(bias_x),
        "bias_m": np.ascontiguousarray(bias_m),
        "ident": np.eye(128, dtype=BF16),
        "sel": np.repeat(np.eye(4, dtype=BF16), 128, axis=1),
        "bn": np.ascontiguousarray(bn),
    }


_NC_CACHE = {}


def _get_nc():
    if "nc" not in _NC_CACHE:
        _NC_CACHE["nc"] = _build_nc()
    return _NC_CACHE["nc"]


def _assemble(results):
    out = np.zeros((B, C2, H, W), dtype=np.float32)
    for c in range(NCORES):
        b, r = c // 2, c % 2
        o = np.asarray(results[c]["out"])     # [2, 128, 2048]
        for ot in range(2):
            out[b, ot * 128:(ot + 1) * 128, HL * r:HL * (r + 1), :] = (
                o[ot].reshape(128, HL, W).astype(np.float32)
            )
    return out


def _run(inputs, trace=False):
    from concourse.bass_utils import run_bass_kernel_spmd
    nc = _get_nc()
    in_maps = [_prep_core_inputs(inputs, c // 2, c % 2) for c in range(NCORES)]
    res = run_bass_kernel_spmd(nc, in_maps, list(range(NCORES)), trace=trace)
    return _assemble(res.results), res


def kernel(**inputs):
    out, _ = _run(inputs, trace=False)
    return out


# revision 28
# speedup vs baseline: 1.5446x; 1.5446x over previous
"""DCNv2 (modulated deformable conv) + BN + SiLU Trainium2 Bass kernel.

Problem: nn_DeformConv_58935541236111
  x[4,256,64,64]: offset/mask conv (3x3, 256->27, +bias) -> clamp/sigmoid ->
  bilinear sampling -> einsum over (C1*KK) with w_dcn -> BN -> SiLU.

Sharding: 8 cores = batch (4) x row-half (2); core computes
out[b, :, 32r:32r+32, :]  (2048 pixels, row-major p = h_loc*64 + w).

v3 design (channel-major, no DMA weight broadcast):
  - x staged as row-pair tokens in SBUF: token (x,y) = 512 bf16 =
    [256ch @ (y,x), 256ch @ (y+1,x)] over a 48-row x 80-col zero-padded
    grid.  Bilinear corners of one sample = tokens idx and idx+48.
  - dma_gather (SBUF source, transpose=True, 1KB tokens): per (k, half)
    one gather of 2048 idxs -> gh[128ch, 4, 2048] channel-major
    (j = row y0/y1 x ct; i<1024 col x0, i>=1024 col x0+1).
  - corner weights: computed per-pixel on 128 partitions (cheap chain),
    staged to DRAM, reloaded as [36, 2, 1024] rows, then broadcast
    across partitions by the PE (ones[1,128] ^T @ w_row[1,1024] into
    PSUM bf16) and copied to SBUF by ACT.  Zero DMA broadcast bytes.
  - lerp: 12 DVE + 2 Pool tensor_tensor ops per (k, half) produce two
    corner-pair partial cols (colA = w00*v00 + w01*v01, colB = w10*v10
    + w11*v11); the final pair-sum is absorbed into a doubled einsum
    (PSUM accumulates colA and colB matmuls).
  - BN+SiLU per pixel-half; store row-major (no permutation).
  - pipeline: 18 (half,k) steps, gather i ahead of compute i-2.
"""

import numpy as np
import ml_dtypes

B, C1, C2, H, W = 4, 256, 256, 64, 64
MAX_OFF = 6.0
BN_EPS = 1e-5

NCORES = 8
HL = 32                 # rows per core
P = HL * W              # 2048 pixels / core
PH = P // 2             # pixels per half
PAD = 8
GRID_Y = HL + 2 * PAD   # 48
GRID_X = W + 2 * PAD    # 80
NTOK = GRID_X * GRID_Y  # 3840 = 128 * 30
NIDX = 2048             # idxs per gather (1024 px * 2 columns)

BF16 = ml_dtypes.bfloat16


def _build_nc():
    import concourse.bacc as bacc
    import concourse.mybir as mybir
    import concourse.tile as tile

    f32 = mybir.dt.float32
    bf16 = mybir.dt.bfloat16
    i16 = mybir.dt.int16
    i32 = mybir.dt.int32
    AF = mybir.ActivationFunctionType
    OP = mybir.AluOpType

    nc = bacc.Bacc("TRN2", target_bir_lowering=False, debug=False)

    x_pairs_d = nc.dram_tensor("x_pairs", [NTOK, 512], bf16, kind="ExternalInput")
    x_conv_d = nc.dram_tensor("x_conv", [2, 128, 34, 66], bf16, kind="ExternalInput")
    w_om_d = nc.dram_tensor("w_om", [9, 2, 128, 27], bf16, kind="ExternalInput")
    w_dcn_d = nc.dram_tensor("w_dcn", [9, 2, 2, 128, 128], bf16, kind="ExternalInput")
    base_y_d = nc.dram_tensor("base_y", [128, 9, 16], f32, kind="ExternalInput")
    base_x_d = nc.dram_tensor("base_x", [128, 9, 16], f32, kind="ExternalInput")
    bias_y_d = nc.dram_tensor("bias_y", [128, 9, 16], f32, kind="ExternalInput")
    bias_x_d = nc.dram_tensor("bias_x", [128, 9, 16], f32, kind="ExternalInput")
    bias_m_d = nc.dram_tensor("bias_m", [128, 9, 16], f32, kind="ExternalInput")
    ident_d = nc.dram_tensor("ident", [128, 128], bf16, kind="ExternalInput")
    sel_d = nc.dram_tensor("sel", [32, 4096], bf16, kind="ExternalInput")
    bn_d = nc.dram_tensor("bn", [4, 128, 2], f32, kind="ExternalInput")
    out_d = nc.dram_tensor("out", [2, 128, P], f32, kind="ExternalOutput")
    # idx staging, read-order layout: [hs][q][k][col][sl][ph]
    stage_d = nc.dram_tensor("stage", [2, 16, 9, 2, 8, 8], i16)

    with tile.TileContext(nc) as tc:
        with (
            tc.tile_pool(name="pers", bufs=1) as pers,
            tc.tile_pool(name="chainp", bufs=1) as chainp,
            tc.tile_pool(name="gpool", bufs=3) as gpool,
            tc.tile_pool(name="lerpp", bufs=1) as lerpp,
            tc.tile_pool(name="colp", bufs=2) as colp,
            tc.tile_pool(name="wsbp", bufs=2) as wsbp,
            tc.tile_pool(name="tpool", bufs=2) as tpool,
            tc.tile_pool(name="outp", bufs=1) as outp,
        ):
            # ---------------- persistent tensors / loads ----------------
            ident = pers.tile([128, 128], bf16)
            nc.scalar.dma_start(ident[:], ident_d[:])
            w_om = pers.tile([128, 9, 2, 27], bf16)
            nc.scalar.dma_start(w_om[:], w_om_d[:].rearrange("k c p o -> p k c o"))
            xc = pers.tile([128, 2, 34, 66], bf16)
            nc.scalar.dma_start(xc[:], x_conv_d[:].rearrange("c p a b -> p c a b"))
            base_y = pers.tile([128, 9, 16], f32)
            nc.sync.dma_start(base_y[:], base_y_d[:])
            base_x = pers.tile([128, 9, 16], f32)
            nc.sync.dma_start(base_x[:], base_x_d[:])
            bias_y = pers.tile([128, 9, 16], f32)
            nc.sync.dma_start(bias_y[:], bias_y_d[:])
            bias_x = pers.tile([128, 9, 16], f32)
            nc.sync.dma_start(bias_x[:], bias_x_d[:])
            bias_m = pers.tile([128, 9, 16], f32)
            nc.sync.dma_start(bias_m[:], bias_m_d[:])
            bn_in = pers.tile([128, 4, 2], f32)
            nc.sync.dma_start(bn_in[:], bn_d[:].rearrange("a p b -> p a b"))
            x_sb = pers.tile([128, 30, 512], bf16)
            nc.sync.dma_start(x_sb[:], x_pairs_d[:].rearrange("(r p) e -> p r e", p=128))
            wd = pers.tile([128, 9, 2, 2, 128], bf16)
            nc.sync.dma_start(wd[:], w_dcn_d[:].rearrange("k c o p q -> p k c o q"))

            bn_s = pers.tile([128, 2], f32)
            bn_o = pers.tile([128, 2], f32)
            wrap_rep = pers.tile([128, 2, 9, 128], i16)
            # [p, hs, k, row(y), col(x), sl] - bf16 corner weights
            w4 = pers.tile([128, 2, 9, 2, 2, 8], bf16)
            idx_all = pers.tile([128, 2, 9, 2, 8], i16)   # [p, hs, k, col, sl]
            # transposed corner weights: rows j = row*16+col*8+sl, free (k, hs, pp)
            w_movT = pers.tile([32, 9, 2, 128], bf16)
            sel = pers.tile([32, 4096], bf16)
            nc.scalar.dma_start(sel[:], sel_d[:])

            def ct_(name, dtype=f32, shape=(128, 9, 8)):
                return chainp.tile(list(shape), dtype, tag=name, name=name)

            def chain_half(hs, omT):
                hsl = slice(hs * 8, hs * 8 + 8)
                omr = omT[:].rearrange("p s o -> p o s")
                dy = ct_("dy"); dx = ct_("dx"); mm = ct_("mm")
                nc.vector.tensor_copy(dy[:], omr[:, 0:18:2, :])
                nc.vector.tensor_copy(dx[:], omr[:, 1:18:2, :])
                nc.vector.tensor_copy(mm[:], omr[:, 18:27, :])
                nc.vector.tensor_tensor(dy[:], dy[:], bias_y[:, :, hsl], OP.add)
                nc.vector.tensor_tensor(dx[:], dx[:], bias_x[:, :, hsl], OP.add)
                nc.vector.tensor_tensor(mm[:], mm[:], bias_m[:, :, hsl], OP.add)
                nc.vector.tensor_scalar(dy[:], dy[:], MAX_OFF, -MAX_OFF, OP.min, OP.max)
                nc.vector.tensor_scalar(dx[:], dx[:], MAX_OFF, -MAX_OFF, OP.min, OP.max)
                py_ = ct_("py_"); px_ = ct_("px_")
                nc.vector.tensor_tensor(py_[:], dy[:], base_y[:, :, hsl], OP.add)
                nc.vector.tensor_tensor(px_[:], dx[:], base_x[:, :, hsl], OP.add)
                iy = ct_("iy", i32); t0 = ct_("t0"); t1 = ct_("t1")
                y0f = ct_("y0f"); x0f = ct_("x0f"); ly = ct_("ly"); lx = ct_("lx")
                # floor via convert + fixup (robust to converter rounding mode)
                nc.vector.tensor_copy(iy[:], py_[:])
                nc.vector.tensor_copy(y0f[:], iy[:])
                nc.vector.tensor_tensor(t0[:], y0f[:], py_[:], OP.is_gt)
                nc.vector.tensor_tensor(y0f[:], y0f[:], t0[:], OP.subtract)
                nc.vector.tensor_tensor(ly[:], py_[:], y0f[:], OP.subtract)
                nc.vector.tensor_copy(iy[:], px_[:])
                nc.vector.tensor_copy(x0f[:], iy[:])
                nc.vector.tensor_tensor(t1[:], x0f[:], px_[:], OP.is_gt)
                nc.vector.tensor_tensor(x0f[:], x0f[:], t1[:], OP.subtract)
                nc.vector.tensor_tensor(lx[:], px_[:], x0f[:], OP.subtract)
                # token index = x0*48 + y0 ; second column at +48
                nc.vector.tensor_scalar(t0[:], x0f[:], float(GRID_Y), None, OP.mult)
                nc.vector.tensor_tensor(t0[:], t0[:], y0f[:], OP.add)
                nc.vector.tensor_copy(idx_all[:, hs, :, 0], t0[:])
                nc.vector.tensor_scalar(t0[:], t0[:], float(GRID_Y), None, OP.add)
                nc.vector.tensor_copy(idx_all[:, hs, :, 1], t0[:])
                # corner weights (mask folded in)
                msk = ct_("msk")
                nc.scalar.activation(msk[:], mm[:], AF.Sigmoid)
                oly = ct_("oly"); olx = ct_("olx")
                nc.vector.tensor_scalar(oly[:], ly[:], -1.0, 1.0, OP.mult, OP.add)
                nc.vector.tensor_scalar(olx[:], lx[:], -1.0, 1.0, OP.mult, OP.add)
                wyt = ct_("wyt"); wyb = ct_("wyb")
                nc.vector.tensor_tensor(wyt[:], oly[:], msk[:], OP.mult)
                nc.vector.tensor_tensor(wyb[:], ly[:], msk[:], OP.mult)
                nc.vector.tensor_tensor(w4[:, hs, :, 0, 0], wyt[:], olx[:], OP.mult)
                nc.vector.tensor_tensor(w4[:, hs, :, 0, 1], wyt[:], lx[:], OP.mult)
                nc.vector.tensor_tensor(w4[:, hs, :, 1, 0], wyb[:], olx[:], OP.mult)
                nc.vector.tensor_tensor(w4[:, hs, :, 1, 1], wyb[:], lx[:], OP.mult)

            def wraps_half(hs):
                # indices: SBUF -> DRAM (read-order) -> 8 replicated reads
                for ph in range(8):
                    eng = nc.sync if ph % 2 == 0 else nc.scalar
                    eng.dma_start(
                        stage_d[hs, :, :, :, :, ph],
                        idx_all[ph * 16:(ph + 1) * 16, hs],
                    )
                for g in range(8):
                    eng = nc.sync if g % 2 == 0 else nc.scalar
                    eng.dma_start(
                        wrap_rep[g * 16:(g + 1) * 16, hs],
                        stage_d[hs].rearrange("q k col sl ph -> q k (col sl ph)"),
                    )

            gh_tiles = {}

            def gather(hs, k):
                gh = gpool.tile([128, 4, NIDX], bf16, tag="gh", name=f"gh{hs}_{k}")
                gh_tiles[(hs, k)] = gh
                nc.gpsimd.dma_gather(
                    gh[:],
                    x_sb[:].rearrange("p r e -> p (r e)"),
                    wrap_rep[:, hs, k],
                    NIDX,
                    NIDX,
                    512,
                    transpose=True,
                    sbuf_tokens_per_rank=128,
                    sbuf_free_dim_per_rank=1024,
                    single_packet=False,
                )

            # ---------------- phase A: conv + chain (psum scope A) -------
            with tc.tile_pool(name="psA", bufs=1, space="PSUM") as psA:
                # BN constants
                tv = chainp.tile([128, 2], f32, tag="tv", name="tv")
                nc.vector.tensor_scalar(tv[:], bn_in[:, 3], BN_EPS, None, OP.add)
                nc.scalar.sqrt(tv[:], tv[:])
                nc.vector.reciprocal(tv[:], tv[:])
                nc.vector.tensor_tensor(bn_s[:], bn_in[:, 0], tv[:], OP.mult)
                nc.vector.tensor_tensor(bn_o[:], bn_in[:, 2], bn_s[:], OP.mult)
                nc.vector.tensor_tensor(bn_o[:], bn_in[:, 1], bn_o[:], OP.subtract)

                def conv_half(hs):
                    om_ps = psA.tile([27, PH], f32, tag="om", name=f"om{hs}")
                    for ky in range(3):
                        for kx in range(3):
                            k = ky * 3 + kx
                            for ctile in range(2):
                                for n in range(2):
                                    nc.tensor.matmul(
                                        om_ps[:, n * 512:(n + 1) * 512],
                                        w_om[:, k, ctile],
                                        xc[:, ctile,
                                           hs * 16 + n * 8 + ky: hs * 16 + n * 8 + ky + 8,
                                           kx: kx + 64],
                                        start=(k == 0 and ctile == 0),
                                        stop=(k == 8 and ctile == 1),
                                    )
                    om_sb = chainp.tile([27, PH], bf16, tag="om_sb", name=f"om_sb{hs}")
                    nc.scalar.copy(om_sb[:], om_ps[:])
                    omT_ps = psA.tile([128, 8 * 28], bf16, tag="omt", name=f"omt{hs}")
                    for c8 in range(8):
                        nc.tensor.transpose(
                            omT_ps[:, c8 * 28:c8 * 28 + 27],
                            om_sb[:, c8 * 128:(c8 + 1) * 128],
                            ident[:27, :27],
                        )
                    omT = chainp.tile([128, 8, 27], f32, tag=f"omT{hs}", name=f"omT{hs}")
                    nc.scalar.copy(
                        omT[:],
                        omT_ps[:].rearrange("p (a b) -> p a b", a=8)[:, :, 0:27],
                    )
                    return omT

                def wt_half(hs):
                    # PE-transpose w4 [128, (row col sl)=32] -> w_movT rows
                    wT_ps = psA.tile([32, 9, 128], bf16, tag=f"wt{hs}",
                                     name=f"wT_ps{hs}")
                    for k in range(9):
                        nc.tensor.transpose(
                            wT_ps[:, k],
                            w4[:, hs, k].rearrange("p a b c -> p (a b c)"),
                            ident[:],
                        )
                    nc.scalar.copy(w_movT[:, :, hs], wT_ps[:])

                # PE warm-up: keep the tensor engine continuously busy so
                # the p-state ramps to full before the offset conv starts
                junk = psA.tile([128, 128], bf16, tag="junk", name="junk")
                for _ in range(40):
                    nc.tensor.transpose(junk[:], ident[:], ident[:])

                omT0 = conv_half(0)
                chain_half(0, omT0)
                wraps_half(0)
                wt_half(0)
                gather(0, 0)
                gather(0, 1)
                omT1 = conv_half(1)
                chain_half(1, omT1)
                wraps_half(1)
                wt_half(1)

            # ---------------- phase B: main loop (psum scope B) ----------
            sched = [(0, k) for k in range(9)] + [(1, k) for k in range(9)]
            with tc.tile_pool(name="psB", bufs=1, space="PSUM") as psB:
                out_ps = [psB.tile([128, PH], f32, tag=f"o{ot}", name=f"out_ps{ot}")
                          for ot in range(2)]

                w_sb_tiles = {}
                tp_tiles = {}

                def wbc(hs, k):
                    # PE: broadcast the 4 corner-weight rows across partitions
                    # (per-corner rounds; 2 rotating psum tiles pipeline the
                    # PE matmuls against the ACT f32->bf16 copies)
                    w_sb = wsbp.tile([128, 4, PH], bf16, tag="wsb",
                                     name=f"wsb{hs}_{k}")
                    w_sb_tiles[(hs, k)] = w_sb
                    for cr in range(4):
                        w_ps = psB.tile([128, PH], f32, tag=f"wps{cr % 2}",
                                        name=f"wps{hs}_{k}_{cr}")
                        r, c = cr // 2, cr % 2
                        for sl in range(8):
                            j = r * 16 + c * 8 + sl
                            nc.tensor.matmul(
                                w_ps[:, sl * 128:(sl + 1) * 128],
                                sel[:, j * 128:(j + 1) * 128],
                                w_movT[:, k, hs],
                                start=True, stop=True)
                        nc.scalar.copy(w_sb[:, cr], w_ps[:])

                def pool_tp(hs, k):
                    # Pool precomputes the w11*v11 products one step ahead
                    gh = gh_tiles[(hs, k)]
                    w_sb = w_sb_tiles[(hs, k)]
                    tp = tpool.tile([128, 2, PH], bf16, tag="tp",
                                    name=f"tp{hs}_{k}")
                    tp_tiles[(hs, k)] = tp
                    for ctile in range(2):
                        nc.gpsimd.tensor_tensor(
                            tp[:, ctile], gh[:, 2 + ctile, PH:2 * PH],
                            w_sb[:, 3], OP.mult)

                def compute(hs, k):
                    gh = gh_tiles.pop((hs, k))
                    w_sb = w_sb_tiles.pop((hs, k))
                    tp = tp_tiles.pop((hs, k))
                    # lerp: colA = w00*v00 + w01*v01, colB = w10*v10 + w11*v11
                    colA = colp.tile([128, 2, PH], bf16, tag="colA",
                                     name=f"colA{hs}_{k}")
                    colB = colp.tile([128, 2, PH], bf16, tag="colB",
                                     name=f"colB{hs}_{k}")
                    ta = lerpp.tile([128, PH], bf16, tag="ta", name=f"ta{hs}_{k}")
                    for ctile in range(2):
                        cA = colA[:, ctile]
                        cB = colB[:, ctile]
                        v00 = gh[:, ctile, 0:PH]
                        v01 = gh[:, ctile, PH:2 * PH]
                        v10 = gh[:, 2 + ctile, 0:PH]
                        nc.vector.tensor_tensor(cA, v00, w_sb[:, 0], OP.mult)
                        nc.vector.tensor_tensor(ta[:], v01, w_sb[:, 1], OP.mult)
                        nc.vector.tensor_tensor(cA, cA, ta[:], OP.add)
                        nc.vector.tensor_tensor(cB, v10, w_sb[:, 2], OP.mult)
                        nc.vector.tensor_tensor(cB, cB, tp[:, ctile], OP.add)
                        for ot in range(2):
                            for n in range(2):
                                sl = slice(n * 512, (n + 1) * 512)
                                nc.tensor.matmul(
                                    out_ps[ot][:, sl],
                                    wd[:, k, ctile, ot],
                                    colA[:, ctile, sl],
                                    start=(k == 0 and ctile == 0),
                                    stop=False,
                                )
                                nc.tensor.matmul(
                                    out_ps[ot][:, sl],
                                    wd[:, k, ctile, ot],
                                    colB[:, ctile, sl],
                                    start=False,
                                    stop=(k == 8 and ctile == 1),
                                )

                def bn_half(hs):
                    for ot in range(2):
                        yv = outp.tile([128, PH], f32, tag="yv", name=f"yv{hs}_{ot}")
                        sg = outp.tile([128, PH], f32, tag="sg", name=f"sg{hs}_{ot}")
                        nc.vector.tensor_scalar(
                            yv[:], out_ps[ot][:],
                            bn_s[:, ot:ot + 1], bn_o[:, ot:ot + 1],
                            OP.mult, OP.add)
                        nc.scalar.activation(sg[:], yv[:], AF.Sigmoid)
                        nc.vector.tensor_tensor(yv[:], yv[:], sg[:], OP.mult)
                        nc.sync.dma_start(out_d[ot, :, hs * PH:(hs + 1) * PH], yv[:])

                # software pipeline: gather(i) | wbc/tp(i-1) | compute(i-2)
                wbc(*sched[0])
                pool_tp(*sched[0])
                wbc(*sched[1])
                for i in range(2, 18):
                    gather(*sched[i])
                    pool_tp(*sched[i - 1])
                    wbc(*sched[i])
                    hs, k = sched[i - 2]
                    compute(hs, k)
                    if k == 8:
                        bn_half(hs)
                pool_tp(*sched[17])
                compute(*sched[16])
                compute(*sched[17])
                bn_half(1)

    nc.compile()
    return nc


def _prep_core_inputs(inputs, b, r):
    x = np.asarray(inputs["x"])
    w_om = np.asarray(inputs["w_om"])
    b_om = np.asarray(inputs["b_om"])
    w_dcn = np.asarray(inputs["w_dcn"])
    h0 = HL * r

    # padded spatial-major grid [49 rows, 80 cols, 256 ch] (extra row so the
    # y+1 half of the last token row reads zeros)
    xp = np.zeros((GRID_Y + 1, GRID_X, 256), dtype=BF16)
    y_lo, y_hi = max(0, h0 - PAD), min(H, h0 + HL + PAD)
    xp[y_lo - (h0 - PAD):y_hi - (h0 - PAD), PAD:PAD + W, :] = (
        x[b][:, y_lo:y_hi, :].transpose(1, 2, 0).astype(BF16)
    )
    pair = np.concatenate([xp[0:GRID_Y], xp[1:GRID_Y + 1]], axis=2)  # [48, 80, 512]
    x_pairs = np.ascontiguousarray(pair.swapaxes(0, 1).reshape(NTOK, 512))

    xcv = np.zeros((256, 34, 66), dtype=BF16)
    r_lo, r_hi = max(0, h0 - 1), min(H, h0 + 33)
    xcv[:, r_lo - (h0 - 1):r_hi - (h0 - 1), 1:65] = x[b][:, r_lo:r_hi, :].astype(BF16)
    x_conv = np.ascontiguousarray(xcv.reshape(2, 128, 34, 66))

    wl = np.zeros((9, 2, 128, 27), dtype=BF16)
    for ky in range(3):
        for kx in range(3):
            k = ky * 3 + kx
            for ctile in range(2):
                wl[k, ctile] = w_om[:, ctile * 128:(ctile + 1) * 128, ky, kx].T.astype(BF16)

    wdl = np.zeros((9, 2, 2, 128, 128), dtype=BF16)
    wr = w_dcn.reshape(C2, C1, 9)
    for k in range(9):
        for ctile in range(2):
            for ot in range(2):
                wdl[k, ctile, ot] = wr[ot * 128:(ot + 1) * 128,
                                       ctile * 128:(ctile + 1) * 128, k].T.astype(BF16)

    # pixel p = fl*128 + part ; h_loc = p//64, w = p%64 (row-major)
    p_ = np.arange(128)[:, None, None]
    k_ = np.arange(9)[None, :, None]
    fl = np.arange(16)[None, None, :]
    pix = fl * 128 + p_
    h_loc = pix // W
    w_pix = pix % W
    ky_ = k_ // 3
    kx_ = k_ % 3
    base_y = np.broadcast_to(h_loc + ky_ - 1 + PAD, (128, 9, 16)).astype(np.float32)
    base_x = np.broadcast_to(w_pix + kx_ - 1 + PAD, (128, 9, 16)).astype(np.float32)
    bias_y = np.broadcast_to(b_om[0:18:2][None, :, None], (128, 9, 16)).astype(np.float32)
    bias_x = np.broadcast_to(b_om[1:18:2][None, :, None], (128, 9, 16)).astype(np.float32)
    bias_m = np.broadcast_to(b_om[18:27][None, :, None], (128, 9, 16)).astype(np.float32)

    bn = np.stack([
        np.asarray(inputs["bn_gamma"]).reshape(2, 128).T,
        np.asarray(inputs["bn_beta"]).reshape(2, 128).T,
        np.asarray(inputs["bn_mean"]).reshape(2, 128).T,
        np.asarray(inputs["bn_var"]).reshape(2, 128).T,
    ], axis=0).astype(np.float32)

    return {
        "x_pairs": x_pairs,
        "x_conv": x_conv,
        "w_om": wl,
        "w_dcn": wdl,
        "base_y": np.ascontiguousarray(base_y),
        "base_x": np.ascontiguousarray(base_x),
        "bias_y": np.ascontiguousarray(bias_y),
        "bias_x": np.ascontiguousarray(bias_x),
        "bias_m": np.ascontiguousarray(bias_m),
        "ident": np.eye(128, dtype=BF16),
        "sel": np.repeat(np.eye(32, dtype=BF16), 128, axis=1),
        "bn": np.ascontiguousarray(bn),
    }


_NC_CACHE = {}


def _get_nc():
    if "nc" not in _NC_CACHE:
        _NC_CACHE["nc"] = _build_nc()
    return _NC_CACHE["nc"]


def _assemble(results):
    out = np.zeros((B, C2, H, W), dtype=np.float32)
    for c in range(NCORES):
        b, r = c // 2, c % 2
        o = np.asarray(results[c]["out"])     # [2, 128, 2048]
        for ot in range(2):
            out[b, ot * 128:(ot + 1) * 128, HL * r:HL * (r + 1), :] = (
                o[ot].reshape(128, HL, W).astype(np.float32)
            )
    return out


def _run(inputs, trace=False):
    from concourse.bass_utils import run_bass_kernel_spmd
    nc = _get_nc()
    in_maps = [_prep_core_inputs(inputs, c // 2, c % 2) for c in range(NCORES)]
    res = run_bass_kernel_spmd(nc, in_maps, list(range(NCORES)), trace=trace)
    return _assemble(res.results), res


def kernel(**inputs):
    out, _ = _run(inputs, trace=False)
    return out
